# revision 17
# baseline (speedup 1.0000x reference)
"""Trainium2 Bass kernel for nn_DiscSeqRNNDecoder.

Free-running GRU decoder with argmax token feedback:
  h0 = x @ W_i2h.T + b_i2h ; i0 = sos_embed
  per step: GRU cell -> logits -> argmax tok -> emb[tok] feedback
Returns (toks [B,L] int32, logits [B,L,V] fp32).

Pure data parallel over 8 NeuronCores (1024 batch rows each).
"Transposed state" layout: h/i stored [feature, batch] so weights are the
matmul stationary operand; logits computed directly in [batch, V] by using
the state slices as the stationary operand. Argmax via DVE max/max_index;
token -> embedding via one-hot matmul (one-hot built from a PE transpose of
the token column + partition-broadcast matmul + iota compare).

Numerics: fp32r matmuls (1 cycle/row, vs 4 for fp32) with 3-pass hi/lo
error compensation: every real operand X is device-split into
Xhi = round_f32r(X), Xlo = X - Xhi (exact), and W@A is computed as
Whi@Ahi + Wlo@Ahi + Whi@Alo accumulated in fp32 PSUM -> ~1.7e-7 relative
error (fp32-faithful). Needed because argmax feedback makes the output
discontinuous in the logits (1-pass fp32r flips ~900 tokens).
"""
import time

import numpy as np

import concourse.bass as bass
from concourse import bacc, mybir
import concourse.tile as tile
from concourse.bass_utils import run_bass_kernel_spmd

F32 = mybir.dt.float32
F32R = mybir.dt.float32r
U32 = mybir.dt.uint32
I32 = mybir.dt.int32

B, L, V, D_IN, H, E = 8192, 32, 256, 256, 512, 256
NCORES = 8
BL = B // NCORES         # 1024 rows per core
BC = 512                 # batch chunk (matmul moving free dim)
NCH = BL // BC           # 2 chunks
NSUB = BC // 128         # 4 subtiles of 128 per chunk
KH = H // 128            # 4
KE = E // 128            # 2
KV = V // 128            # 2
KD = D_IN // 128         # 2
MRZ = 2 * H // 128       # 8 (r,z gate tiles)
MN = H // 128            # 4 (n gate tiles)

Sig = mybir.ActivationFunctionType.Sigmoid
Tanh = mybir.ActivationFunctionType.Tanh
Ident = mybir.ActivationFunctionType.Identity


DBG_TAPS = {"on": False}


def build_program(n_steps=L, bench_iters=None):
    nc = bacc.Bacc("TRN2", target_bir_lowering=False, debug=False)

    def din(name, shape, dt=F32):
        return nc.dram_tensor(name, shape, dt, kind="ExternalInput").ap()

    d_xT = din("xT", [D_IN, BL])
    d_wi2hT = din("wi2hT", [D_IN, H])
    d_M = din("M", [V, 3 * H])
    d_whhT = din("whhT", [H, 3 * H])
    d_woutT = din("woutT", [H, V])
    d_brz = din("b_rz", [128, MRZ])
    d_brz0 = din("b_rz0", [128, MRZ])
    d_bn0 = din("b_n0", [128, MN])
    d_bhhn = din("b_hhn", [128, MN])
    d_bihn = din("b_ihn", [128, MN])
    d_bi2h = din("b_i2h", [128, KH])
    d_bout = din("b_out_bc", [128, V])
    d_iota = din("iota", [128, KV])
    d_ones = din("ones", [1, 128], F32R)
    d_ident = din("ident", [128, 128])

    _taps = {}

    def tapout(nm, shape):
        return nc.dram_tensor(f"dbg_{nm}", shape, F32, kind="ExternalOutput").ap()

    d_toks = nc.dram_tensor("toks", [BL, L], I32, kind="ExternalOutput").ap()
    d_logits = nc.dram_tensor("logits", [BL, L, V], F32, kind="ExternalOutput").ap()

    with tile.TileContext(nc) as tc:
        with (
            tc.tile_pool(name="wpool", bufs=1) as wpool,      # persistent weights/state
            tc.tile_pool(name="pg", bufs=3, space="PSUM") as pg,
            tc.tile_pool(name="pl", bufs=2, space="PSUM") as pl,
            tc.tile_pool(name="pm", bufs=2, space="PSUM") as pm,
        ):
            def split_into(pool, dram_ap, rows, cols, kname, stgpool):
                his, los = [], []
                for k in range(rows // 128):
                    stg = stgpool.tile([128, cols], F32, tag="stg", bufs=2)
                    nc.sync.dma_start(stg[:], dram_ap[k * 128:(k + 1) * 128, :])
                    hi = pool.tile([128, cols], F32R, tag=f"{kname}hi{k}", bufs=1, name=f"{kname}hi{k}")
                    lo = pool.tile([128, cols], F32R, tag=f"{kname}lo{k}", bufs=1, name=f"{kname}lo{k}")
                    nc.vector.tensor_copy(hi[:], stg[:])
                    nc.vector.tensor_sub(lo[:], stg[:], hi[:])
                    his.append(hi)
                    los.append(lo)
                return his, los

            # persistent state (per chunk) + persistent weights
            hhi = [[wpool.tile([128, BC], F32R, tag=f"hhi{c}{j}", name=f"hhi{c}{j}") for j in range(KH)]
                   for c in range(NCH)]
            hlo = [[wpool.tile([128, BC], F32R, tag=f"hlo{c}{j}", name=f"hlo{c}{j}") for j in range(KH)]
                   for c in range(NCH)]
            ohst = [[wpool.tile([128, BC], F32R, tag=f"oh{c}{v}", name=f"oh{c}{v}")
                     for v in range(KV)] for c in range(NCH)]
            toks_buf = [wpool.tile([128, L], I32, tag=f"tk{s}", name=f"tk{s}")
                        for s in range(BL // 128)]
            for tb in toks_buf:
                nc.gpsimd.memset(tb[:], 0)

            t_brz = wpool.tile([128, MRZ], F32); nc.sync.dma_start(t_brz[:], d_brz)
            t_brz0 = wpool.tile([128, MRZ], F32); nc.sync.dma_start(t_brz0[:], d_brz0)
            t_bn0 = wpool.tile([128, MN], F32); nc.sync.dma_start(t_bn0[:], d_bn0)
            t_bhhn = wpool.tile([128, MN], F32); nc.sync.dma_start(t_bhhn[:], d_bhhn)
            t_bihn = wpool.tile([128, MN], F32); nc.sync.dma_start(t_bihn[:], d_bihn)
            t_bi2h = wpool.tile([128, KH], F32); nc.sync.dma_start(t_bi2h[:], d_bi2h)
            t_bout = wpool.tile([128, V], F32); nc.sync.dma_start(t_bout[:], d_bout)
            t_iota = wpool.tile([128, KV], F32); nc.sync.dma_start(t_iota[:], d_iota)
            t_ones = wpool.tile([1, 128], F32R); nc.sync.dma_start(t_ones[:], d_ones)
            t_ident = wpool.tile([128, 128], F32); nc.sync.dma_start(t_ident[:], d_ident)

            def mm3(psum, w_hi_t, w_lo_t, a_hi_t, a_lo_t, first, last):
                """3-pass split product: whi@ahi + wlo@ahi + whi@alo (accumulate)"""
                nc.tensor.matmul(psum, w_hi_t, a_hi_t, start=first, stop=False)
                nc.tensor.matmul(psum, w_lo_t, a_hi_t, start=False, stop=False)
                nc.tensor.matmul(psum, w_hi_t, a_lo_t, start=False, stop=last)

            def store_split(dst_hi, dst_lo, src):
                nc.vector.tensor_copy(dst_hi, src)
                nc.vector.tensor_sub(dst_lo, src, dst_hi)

            with tc.tile_pool(name="wmain", bufs=1) as wmain:
                # -------- setup pool: closed before the decode loop --------
                with tc.tile_pool(name="setup", bufs=1) as setup:
                    m_hi, m_lo = split_into(wmain, d_M, V, 3 * H, "m", setup)
                    whh_hi, whh_lo = split_into(wmain, d_whhT, H, 3 * H, "whh", setup)
                    wout_hi, wout_lo = split_into(wmain, d_woutT, H, V, "wout", setup)
                    wi2h_hi, wi2h_lo = split_into(setup, d_wi2hT, D_IN, H,
                                                  "wi2h", setup)
                    x_hi, x_lo = split_into(setup, d_xT, D_IN, BL, "x", setup)
                    for c in range(NCH):
                        cs = slice(c * BC, (c + 1) * BC)
                        for m in range(KH):
                            ms = slice(m * 128, (m + 1) * 128)
                            p = pg.tile([128, BC], F32, tag="pg")
                            for k in range(KD):
                                mm3(p[:], wi2h_hi[k][:, ms], wi2h_lo[k][:, ms],
                                    x_hi[k][:, cs], x_lo[k][:, cs],
                                    k == 0, k == KD - 1)
                            h0 = setup.tile([128, BC], F32, tag="h0", bufs=2)
                            nc.scalar.activation(h0[:], p[:], Ident,
                                                 bias=t_bi2h[:, m:m + 1])
                            store_split(hhi[c][m][:], hlo[c][m][:], h0[:])


                # -------- steady-state pools + decode loop --------
                with (
                    tc.tile_pool(name="gw", bufs=2) as gw,
                    tc.tile_pool(name="lgp", bufs=6) as lgp,
                    tc.tile_pool(name="tiny", bufs=8) as tiny,
                    tc.tile_pool(name="ohp", bufs=2) as ohp,
                ):
                    def emit_step(t):
                        rz = [[None] * MRZ for _ in range(NCH)]
                        for c in range(NCH):
                            for m in range(MRZ):
                                ms = slice(m * 128, (m + 1) * 128)
                                p = pg.tile([128, BC], F32, tag="pg")
                                for k in range(KH):
                                    mm3(p[:], whh_hi[k][:, ms], whh_lo[k][:, ms],
                                        hhi[c][k][:], hlo[c][k][:], k == 0,
                                        t == 0 and k == KH - 1)
                                if t > 0:
                                    for k in range(KV):
                                        last = k == KV - 1
                                        nc.tensor.matmul(p[:], m_hi[k][:, ms],
                                                         ohst[c][k][:],
                                                         start=False, stop=False)
                                        nc.tensor.matmul(p[:], m_lo[k][:, ms],
                                                         ohst[c][k][:],
                                                         start=False, stop=last)
                                g = gw.tile([128, BC], F32, tag="rz", bufs=10)
                                brz = t_brz0 if t == 0 else t_brz
                                nc.scalar.activation(g[:], p[:], Sig,
                                                     bias=brz[:, m:m + 1])
                                rz[c][m] = g
                                if DBG_TAPS["on"] and t == 0 and c == 0 and m in (0, MN):
                                    nc.sync.dma_start(tapout(f"rz{m}", [128, BC]), g[:])
                            hn_tiles = []
                            for m in range(MN):
                                ms = slice(2 * H + m * 128, 2 * H + (m + 1) * 128)
                                ms2 = ms
                                phn = pg.tile([128, BC], F32, tag="pg")
                                for k in range(KH):
                                    mm3(phn[:], whh_hi[k][:, ms], whh_lo[k][:, ms],
                                        hhi[c][k][:], hlo[c][k][:],
                                        k == 0, k == KH - 1)
                                t2 = gw.tile([128, BC], F32, tag="tmp", bufs=5)
                                nc.vector.scalar_tensor_tensor(
                                    t2[:], phn[:], t_bhhn[:, m:m + 1], rz[c][m][:],
                                    mybir.AluOpType.add, mybir.AluOpType.mult)
                                if t > 0:
                                    pin = pg.tile([128, BC], F32, tag="pg")
                                    for k in range(KV):
                                        nc.tensor.matmul(pin[:], m_hi[k][:, ms2],
                                                         ohst[c][k][:],
                                                         start=(k == 0), stop=False)
                                        nc.tensor.matmul(pin[:], m_lo[k][:, ms2],
                                                         ohst[c][k][:], start=False,
                                                         stop=(k == KV - 1))
                                    s_ = gw.tile([128, BC], F32, tag="tmp", bufs=5)
                                    nc.vector.tensor_add(s_[:], t2[:], pin[:])
                                    n_ = gw.tile([128, BC], F32, tag="tmp", bufs=5)
                                    nc.scalar.activation(n_[:], s_[:], Tanh,
                                                         bias=t_bihn[:, m:m + 1])
                                else:
                                    n_ = gw.tile([128, BC], F32, tag="tmp", bufs=5)
                                    nc.scalar.activation(n_[:], t2[:], Tanh,
                                                         bias=t_bn0[:, m:m + 1])
                                d_ = gw.tile([128, BC], F32, tag="tmp", bufs=5)
                                nc.vector.tensor_sub(d_[:], hhi[c][m][:], n_[:])
                                nc.vector.tensor_add(d_[:], d_[:], hlo[c][m][:])
                                e_ = gw.tile([128, BC], F32, tag="tmp", bufs=5)
                                nc.vector.tensor_mul(e_[:], rz[c][MN + m][:], d_[:])
                                hn = gw.tile([128, BC], F32, tag="hn", bufs=7)
                                nc.vector.tensor_add(hn[:], n_[:], e_[:])
                                hn_tiles.append(hn)
                            # deferred: all gate matmuls of this chunk read the
                            # OLD h; only now overwrite the state tiles
                            for m in range(MN):
                                store_split(hhi[c][m][:], hlo[c][m][:],
                                            hn_tiles[m][:])
                        lg_tiles = {}
                        for c in range(NCH):
                            for s in range(NSUB):
                                ss = slice(s * 128, (s + 1) * 128)
                                sub = c * NSUB + s
                                plg = pl.tile([128, V], F32, tag="pl")
                                for k in range(KH):
                                    mm3(plg[:], hhi[c][k][:, ss], hlo[c][k][:, ss],
                                        wout_hi[k][:], wout_lo[k][:],
                                        k == 0, k == KH - 1)
                                lg = lgp.tile([128, V], F32, tag="lg", bufs=10)
                                nc.vector.tensor_add(lg[:], plg[:], t_bout[:])
                                nc.sync.dma_start(
                                    d_logits[sub * 128:(sub + 1) * 128, t, :], lg[:])
                                lg_tiles[(c, s)] = lg
                        for c in range(NCH):
                            tokrow = ohp.tile([1, BC], F32R, tag="tokrow", bufs=2)
                            for s in range(NSUB):
                                ss = slice(s * 128, (s + 1) * 128)
                                sub = c * NSUB + s
                                lg = lg_tiles[(c, s)]
                                mx = tiny.tile([128, 8], F32, tag="mx")
                                ix = tiny.tile([128, 8], U32, tag="ix")
                                nc.vector.max(mx[:], lg[:])
                                nc.vector.max_index(ix[:], mx[:], lg[:])
                                nc.vector.tensor_copy(toks_buf[sub][:, t:t + 1],
                                                      ix[:, 0:1])
                                tf = tiny.tile([128, 1], F32, tag="tf")
                                nc.vector.tensor_copy(tf[:], ix[:, 0:1])
                                ptT = pm.tile([1, 128], F32, tag="pm")
                                nc.tensor.transpose(ptT[:], tf[:], t_ident[:])
                                nc.scalar.copy(tokrow[0:1, ss], ptT[:])
                            pbc = pm.tile([128, BC], F32, tag="pm")
                            nc.tensor.matmul(pbc[:], t_ones[:], tokrow[:],
                                             start=True, stop=True)
                            for v in range(KV):
                                nc.vector.tensor_scalar(
                                    ohst[c][v][:], pbc[:], t_iota[:, v:v + 1], None,
                                    mybir.AluOpType.is_equal)

                    if bench_iters is None:
                        for t in range(n_steps):
                            emit_step(t)
                    else:
                        emit_step(0)
                        with tc.For_i(0, bench_iters, 1):
                            emit_step(1)

                    for s in range(BL // 128):
                        nc.sync.dma_start(d_toks[s * 128:(s + 1) * 128, :],
                                          toks_buf[s][:])

    nc.dbg_names = {
        "hhi": [[t.tensor.name for t in row] for row in hhi],
        "hlo": [[t.tensor.name for t in row] for row in hlo],
        "oh": [[t.tensor.name for t in row] for row in ohst],
    }
    nc.compile()
    return nc


def make_in_maps(inputs):
    x = np.ascontiguousarray(np.asarray(inputs["x"], dtype=np.float32))
    emb = np.asarray(inputs["emb"], dtype=np.float32)
    sos = np.asarray(inputs["sos_embed"], dtype=np.float32)
    W_i2h = np.asarray(inputs["W_i2h"], dtype=np.float32)
    b_i2h = np.asarray(inputs["b_i2h"], dtype=np.float32)
    W_ih = np.asarray(inputs["W_ih"], dtype=np.float32)
    W_hh = np.asarray(inputs["W_hh"], dtype=np.float32)
    b_ih = np.asarray(inputs["b_ih"], dtype=np.float32)
    b_hh = np.asarray(inputs["b_hh"], dtype=np.float32)
    W_out = np.asarray(inputs["W_out"], dtype=np.float32)
    b_out = np.asarray(inputs["b_out"], dtype=np.float32)

    b_rz = np.ascontiguousarray((b_ih + b_hh)[:2 * H].reshape(MRZ, 128).T)
    b_hhn = np.ascontiguousarray(b_hh[2 * H:].reshape(MN, 128).T)
    b_ihn = np.ascontiguousarray(b_ih[2 * H:].reshape(MN, 128).T)
    b_i2h_t = np.ascontiguousarray(b_i2h.reshape(KH, 128).T)
    b_out_bc = np.ascontiguousarray(np.broadcast_to(b_out, (128, V)))
    iota = np.ascontiguousarray(
        np.arange(V, dtype=np.float32).reshape(KV, 128).T)
    ones = np.ones((1, 128), dtype=np.float32)
    ident = np.eye(128, dtype=np.float32)

    M32 = np.ascontiguousarray((emb @ W_ih.T).astype(np.float32))
    g0 = ((sos[None, :] @ W_ih.T)[0] + b_ih).astype(np.float32)
    b_rz0 = np.ascontiguousarray((g0[:2 * H] + b_hh[:2 * H]).reshape(MRZ, 128).T)
    b_n0 = np.ascontiguousarray(g0[2 * H:].reshape(MN, 128).T)
    common = {
        "wi2hT": np.ascontiguousarray(W_i2h.T),
        "M": M32,
        "whhT": np.ascontiguousarray(W_hh.T),
        "woutT": np.ascontiguousarray(W_out.T),
        "b_rz": b_rz, "b_rz0": b_rz0, "b_n0": b_n0,
        "b_hhn": b_hhn, "b_ihn": b_ihn, "b_i2h": b_i2h_t,
        "b_out_bc": b_out_bc, "iota": iota,
        "ones": ones, "ident": ident,
    }
    in_maps = []
    for c in range(NCORES):
        m = dict(common)
        m["xT"] = np.ascontiguousarray(x[c * BL:(c + 1) * BL].T)
        in_maps.append(m)
    return in_maps


_cached = {}


def kernel(**inputs):
    if "prog" not in _cached:
        _cached["prog"] = build_program()
    nc = _cached["prog"]
    in_maps = make_in_maps(inputs)
    res = run_bass_kernel_spmd(nc, in_maps, core_ids=list(range(NCORES)))
    toks = np.concatenate([r["toks"] for r in res.results], axis=0)
    logits = np.concatenate([r["logits"] for r in res.results], axis=0)
    return toks.astype(np.int32), logits.astype(np.float32)


if __name__ == "__main__":
    t0 = time.time()
    build_program()
    print(f"build+compile: {time.time()-t0:.1f}s")


# revision 21
# speedup vs baseline: 1.2905x; 1.2905x over previous
"""Trainium2 Bass kernel for nn_DiscSeqRNNDecoder.

Free-running GRU decoder with argmax token feedback:
  h0 = x @ W_i2h.T + b_i2h ; i0 = sos_embed
  per step: GRU cell -> logits -> argmax tok -> emb[tok] feedback
Returns (toks [B,L] int32, logits [B,L,V] fp32).

Pure data parallel over 8 NeuronCores (1024 batch rows each).
"Transposed state" layout: h/i stored [feature, batch] so weights are the
matmul stationary operand; logits computed directly in [batch, V] by using
the state slices as the stationary operand. Argmax via DVE max/max_index;
token -> embedding via one-hot matmul (one-hot built from a PE transpose of
the token column + partition-broadcast matmul + iota compare).

Numerics: fp32r matmuls (1 cycle/row, vs 4 for fp32) with 3-pass hi/lo
error compensation: every real operand X is device-split into
Xhi = round_f32r(X), Xlo = X - Xhi (exact), and W@A is computed as
Whi@Ahi + Wlo@Ahi + Whi@Alo accumulated in fp32 PSUM -> ~1.7e-7 relative
error (fp32-faithful). Needed because argmax feedback makes the output
discontinuous in the logits (1-pass fp32r flips ~900 tokens).
"""
import time

import numpy as np

import concourse.bass as bass
from concourse import bacc, mybir
import concourse.tile as tile
from concourse.bass_utils import run_bass_kernel_spmd

F32 = mybir.dt.float32
F32R = mybir.dt.float32r
U32 = mybir.dt.uint32
I32 = mybir.dt.int32

B, L, V, D_IN, H, E = 8192, 32, 256, 256, 512, 256
NCORES = 8
BL = B // NCORES         # 1024 rows per core
BC = 512                 # batch chunk (matmul moving free dim)
NCH = BL // BC           # 2 chunks
NSUB = BC // 128         # 4 subtiles of 128 per chunk
KH = H // 128            # 4
KE = E // 128            # 2
KV = V // 128            # 2
KD = D_IN // 128         # 2
MRZ = 2 * H // 128       # 8 (r,z gate tiles)
MN = H // 128            # 4 (n gate tiles)

Sig = mybir.ActivationFunctionType.Sigmoid
Tanh = mybir.ActivationFunctionType.Tanh
Ident = mybir.ActivationFunctionType.Identity


DBG_TAPS = {"on": False}


def build_program(n_steps=L, bench_iters=None):
    nc = bacc.Bacc("TRN2", target_bir_lowering=False, debug=False)

    def din(name, shape, dt=F32):
        return nc.dram_tensor(name, shape, dt, kind="ExternalInput").ap()

    d_xT = din("xT", [D_IN, BL])
    d_wi2hT = din("wi2hT", [D_IN, H])
    d_M = din("M", [V, 3 * H])
    d_whhT = din("whhT", [H, 3 * H])
    d_woutT = din("woutT", [H, V])
    d_brz = din("b_rz", [128, MRZ])
    d_brz0 = din("b_rz0", [128, MRZ])
    d_bn0 = din("b_n0", [128, MN])
    d_bhhn = din("b_hhn", [128, MN])
    d_bihn = din("b_ihn", [128, MN])
    d_bi2h = din("b_i2h", [128, KH])
    d_bout = din("b_out_bc", [128, V])
    d_iota = din("iota", [128, KV])
    d_ones = din("ones", [1, 128], F32R)
    d_ident = din("ident", [128, 128])

    _taps = {}

    def tapout(nm, shape):
        return nc.dram_tensor(f"dbg_{nm}", shape, F32, kind="ExternalOutput").ap()

    d_toks = nc.dram_tensor("toks", [BL, L], I32, kind="ExternalOutput").ap()
    d_logits = nc.dram_tensor("logits", [BL, L, V], F32, kind="ExternalOutput").ap()

    with tile.TileContext(nc) as tc:
        with (
            tc.tile_pool(name="wpool", bufs=1) as wpool,      # persistent weights/state
            tc.tile_pool(name="pg", bufs=5, space="PSUM") as pg,
            tc.tile_pool(name="pl", bufs=2, space="PSUM") as pl,
            tc.tile_pool(name="pm", bufs=1, space="PSUM") as pm,
        ):
            def split_into(pool, dram_ap, rows, cols, kname, stgpool):
                his, los = [], []
                for k in range(rows // 128):
                    stg = stgpool.tile([128, cols], F32, tag="stg", bufs=2)
                    nc.sync.dma_start(stg[:], dram_ap[k * 128:(k + 1) * 128, :])
                    hi = pool.tile([128, cols], F32R, tag=f"{kname}hi{k}", bufs=1, name=f"{kname}hi{k}")
                    lo = pool.tile([128, cols], F32R, tag=f"{kname}lo{k}", bufs=1, name=f"{kname}lo{k}")
                    nc.vector.tensor_copy(hi[:], stg[:])
                    nc.vector.tensor_sub(lo[:], stg[:], hi[:])
                    his.append(hi)
                    los.append(lo)
                return his, los

            # persistent state (per chunk) + persistent weights
            hhi = [[wpool.tile([128, BC], F32R, tag=f"hhi{c}{j}", name=f"hhi{c}{j}") for j in range(KH)]
                   for c in range(NCH)]
            hlo = [[wpool.tile([128, BC], F32R, tag=f"hlo{c}{j}", name=f"hlo{c}{j}") for j in range(KH)]
                   for c in range(NCH)]
            ohst = [[wpool.tile([128, BC], F32R, tag=f"oh{c}{v}", name=f"oh{c}{v}")
                     for v in range(KV)] for c in range(NCH)]
            toks_buf = [wpool.tile([128, L], I32, tag=f"tk{s}", name=f"tk{s}")
                        for s in range(BL // 128)]
            for tb in toks_buf:
                nc.gpsimd.memset(tb[:], 0)

            t_brz = wpool.tile([128, MRZ], F32); nc.sync.dma_start(t_brz[:], d_brz)
            t_brz0 = wpool.tile([128, MRZ], F32); nc.sync.dma_start(t_brz0[:], d_brz0)
            t_bn0 = wpool.tile([128, MN], F32); nc.sync.dma_start(t_bn0[:], d_bn0)
            t_bhhn = wpool.tile([128, MN], F32); nc.sync.dma_start(t_bhhn[:], d_bhhn)
            t_bihn = wpool.tile([128, MN], F32); nc.sync.dma_start(t_bihn[:], d_bihn)
            t_bi2h = wpool.tile([128, KH], F32); nc.sync.dma_start(t_bi2h[:], d_bi2h)
            t_bout = wpool.tile([128, V], F32); nc.sync.dma_start(t_bout[:], d_bout)
            t_iota = wpool.tile([128, KV], F32); nc.sync.dma_start(t_iota[:], d_iota)
            t_ones = wpool.tile([1, 128], F32R); nc.sync.dma_start(t_ones[:], d_ones)
            t_ident = wpool.tile([128, 128], F32); nc.sync.dma_start(t_ident[:], d_ident)

            def mm3(psum, w_hi_t, w_lo_t, a_hi_t, a_lo_t, first, last):
                """3-pass split product: whi@ahi + wlo@ahi + whi@alo (accumulate)"""
                nc.tensor.matmul(psum, w_hi_t, a_hi_t, start=first, stop=False)
                nc.tensor.matmul(psum, w_lo_t, a_hi_t, start=False, stop=False)
                nc.tensor.matmul(psum, w_hi_t, a_lo_t, start=False, stop=last)

            def store_split(dst_hi, dst_lo, src):
                nc.vector.tensor_copy(dst_hi, src)
                nc.vector.tensor_sub(dst_lo, src, dst_hi)

            with tc.tile_pool(name="wmain", bufs=1) as wmain:
                # -------- setup pool: closed before the decode loop --------
                with tc.tile_pool(name="setup", bufs=1) as setup:
                    m_hi, m_lo = split_into(wmain, d_M, V, 3 * H, "m", setup)
                    whh_hi, whh_lo = split_into(wmain, d_whhT, H, 3 * H, "whh", setup)
                    wout_hi, wout_lo = split_into(wmain, d_woutT, H, V, "wout", setup)
                    wi2h_hi, wi2h_lo = split_into(setup, d_wi2hT, D_IN, H,
                                                  "wi2h", setup)
                    x_hi, x_lo = split_into(setup, d_xT, D_IN, BL, "x", setup)
                    for c in range(NCH):
                        cs = slice(c * BC, (c + 1) * BC)
                        for m in range(KH):
                            ms = slice(m * 128, (m + 1) * 128)
                            p = pg.tile([128, BC], F32, tag="pg")
                            for k in range(KD):
                                mm3(p[:], wi2h_hi[k][:, ms], wi2h_lo[k][:, ms],
                                    x_hi[k][:, cs], x_lo[k][:, cs],
                                    k == 0, k == KD - 1)
                            h0 = setup.tile([128, BC], F32, tag="h0", bufs=2)
                            nc.scalar.activation(h0[:], p[:], Ident,
                                                 bias=t_bi2h[:, m:m + 1])
                            store_split(hhi[c][m][:], hlo[c][m][:], h0[:])


                # -------- steady-state pools + decode loop --------
                with (
                    tc.tile_pool(name="gw", bufs=2) as gw,
                    tc.tile_pool(name="lgp", bufs=6) as lgp,
                    tc.tile_pool(name="tiny", bufs=8) as tiny,
                    tc.tile_pool(name="ohp", bufs=2) as ohp,
                ):
                    def emit_step(t):
                        rz = [[None] * MRZ for _ in range(NCH)]
                        for c in range(NCH):
                            for m in range(MRZ):
                                ms = slice(m * 128, (m + 1) * 128)
                                p = pg.tile([128, BC], F32, tag="pg")
                                for k in range(KH):
                                    mm3(p[:], whh_hi[k][:, ms], whh_lo[k][:, ms],
                                        hhi[c][k][:], hlo[c][k][:], k == 0,
                                        t == 0 and k == KH - 1)
                                if t > 0:
                                    for k in range(KV):
                                        last = k == KV - 1
                                        nc.tensor.matmul(p[:], m_hi[k][:, ms],
                                                         ohst[c][k][:],
                                                         start=False, stop=False)
                                        nc.tensor.matmul(p[:], m_lo[k][:, ms],
                                                         ohst[c][k][:],
                                                         start=False, stop=last)
                                g = gw.tile([128, BC], F32, tag="rz", bufs=10)
                                brz = t_brz0 if t == 0 else t_brz
                                nc.scalar.activation(g[:], p[:], Sig,
                                                     bias=brz[:, m:m + 1])
                                rz[c][m] = g
                                if DBG_TAPS["on"] and t == 0 and c == 0 and m in (0, MN):
                                    nc.sync.dma_start(tapout(f"rz{m}", [128, BC]), g[:])
                            hn_tiles = []
                            for m in range(MN):
                                ms = slice(2 * H + m * 128, 2 * H + (m + 1) * 128)
                                ms2 = ms
                                phn = pg.tile([128, BC], F32, tag="pg")
                                for k in range(KH):
                                    mm3(phn[:], whh_hi[k][:, ms], whh_lo[k][:, ms],
                                        hhi[c][k][:], hlo[c][k][:],
                                        k == 0, k == KH - 1)
                                t2 = gw.tile([128, BC], F32, tag="tmp", bufs=5)
                                nc.vector.scalar_tensor_tensor(
                                    t2[:], phn[:], t_bhhn[:, m:m + 1], rz[c][m][:],
                                    mybir.AluOpType.add, mybir.AluOpType.mult)
                                if t > 0:
                                    pin = pg.tile([128, BC], F32, tag="pg")
                                    for k in range(KV):
                                        nc.tensor.matmul(pin[:], m_hi[k][:, ms2],
                                                         ohst[c][k][:],
                                                         start=(k == 0), stop=False)
                                        nc.tensor.matmul(pin[:], m_lo[k][:, ms2],
                                                         ohst[c][k][:], start=False,
                                                         stop=(k == KV - 1))
                                    s_ = gw.tile([128, BC], F32, tag="tmp", bufs=5)
                                    nc.vector.tensor_add(s_[:], t2[:], pin[:])
                                    n_ = gw.tile([128, BC], F32, tag="tmp", bufs=5)
                                    nc.scalar.activation(n_[:], s_[:], Tanh,
                                                         bias=t_bihn[:, m:m + 1])
                                else:
                                    n_ = gw.tile([128, BC], F32, tag="tmp", bufs=5)
                                    nc.scalar.activation(n_[:], t2[:], Tanh,
                                                         bias=t_bn0[:, m:m + 1])
                                d_ = gw.tile([128, BC], F32, tag="tmp", bufs=5)
                                nc.vector.tensor_sub(d_[:], hhi[c][m][:], n_[:])
                                nc.vector.tensor_add(d_[:], d_[:], hlo[c][m][:])
                                e_ = gw.tile([128, BC], F32, tag="tmp", bufs=5)
                                nc.vector.tensor_mul(e_[:], rz[c][MN + m][:], d_[:])
                                hn = gw.tile([128, BC], F32, tag="hn", bufs=7)
                                nc.vector.tensor_add(hn[:], n_[:], e_[:])
                                hn_tiles.append(hn)
                            # deferred: all gate matmuls of this chunk read the
                            # OLD h; only now overwrite the state tiles
                            for m in range(MN):
                                store_split(hhi[c][m][:], hlo[c][m][:],
                                            hn_tiles[m][:])
                        lg_tiles = {}
                        for c in range(NCH):
                            for s in range(NSUB):
                                ss = slice(s * 128, (s + 1) * 128)
                                sub = c * NSUB + s
                                plg = pl.tile([128, V], F32, tag="pl")
                                for k in range(KH):
                                    mm3(plg[:], hhi[c][k][:, ss], hlo[c][k][:, ss],
                                        wout_hi[k][:], wout_lo[k][:],
                                        k == 0, k == KH - 1)
                                lg = lgp.tile([128, V], F32, tag="lg", bufs=10)
                                nc.vector.tensor_add(lg[:], plg[:], t_bout[:])
                                nc.sync.dma_start(
                                    d_logits[sub * 128:(sub + 1) * 128, t, :], lg[:])
                                lg_tiles[(c, s)] = lg
                        for c in range(NCH):
                            tokrow = ohp.tile([1, BC], F32R, tag="tokrow", bufs=2)
                            for s in range(NSUB):
                                ss = slice(s * 128, (s + 1) * 128)
                                sub = c * NSUB + s
                                lg = lg_tiles[(c, s)]
                                mx = tiny.tile([128, 8], F32, tag="mx")
                                ix = tiny.tile([128, 8], U32, tag="ix")
                                nc.vector.max(mx[:], lg[:])
                                nc.vector.max_index(ix[:], mx[:], lg[:])
                                nc.vector.tensor_copy(toks_buf[sub][:, t:t + 1],
                                                      ix[:, 0:1])
                                tf = tiny.tile([128, 1], F32, tag="tf")
                                nc.vector.tensor_copy(tf[:], ix[:, 0:1])
                                ptT = pm.tile([1, 128], F32, tag="pm")
                                nc.tensor.transpose(ptT[:], tf[:], t_ident[:])
                                nc.scalar.copy(tokrow[0:1, ss], ptT[:])
                            pbc = pm.tile([128, BC], F32, tag="pm")
                            nc.tensor.matmul(pbc[:], t_ones[:], tokrow[:],
                                             start=True, stop=True)
                            for v in range(KV):
                                nc.vector.tensor_scalar(
                                    ohst[c][v][:], pbc[:], t_iota[:, v:v + 1], None,
                                    mybir.AluOpType.is_equal)

                    if bench_iters is None:
                        for t in range(n_steps):
                            emit_step(t)
                    else:
                        emit_step(0)
                        with tc.For_i(0, bench_iters, 1):
                            emit_step(1)

                    for s in range(BL // 128):
                        nc.sync.dma_start(d_toks[s * 128:(s + 1) * 128, :],
                                          toks_buf[s][:])

    nc.dbg_names = {
        "hhi": [[t.tensor.name for t in row] for row in hhi],
        "hlo": [[t.tensor.name for t in row] for row in hlo],
        "oh": [[t.tensor.name for t in row] for row in ohst],
    }
    nc.compile()
    return nc


def make_in_maps(inputs):
    x = np.ascontiguousarray(np.asarray(inputs["x"], dtype=np.float32))
    emb = np.asarray(inputs["emb"], dtype=np.float32)
    sos = np.asarray(inputs["sos_embed"], dtype=np.float32)
    W_i2h = np.asarray(inputs["W_i2h"], dtype=np.float32)
    b_i2h = np.asarray(inputs["b_i2h"], dtype=np.float32)
    W_ih = np.asarray(inputs["W_ih"], dtype=np.float32)
    W_hh = np.asarray(inputs["W_hh"], dtype=np.float32)
    b_ih = np.asarray(inputs["b_ih"], dtype=np.float32)
    b_hh = np.asarray(inputs["b_hh"], dtype=np.float32)
    W_out = np.asarray(inputs["W_out"], dtype=np.float32)
    b_out = np.asarray(inputs["b_out"], dtype=np.float32)

    b_rz = np.ascontiguousarray((b_ih + b_hh)[:2 * H].reshape(MRZ, 128).T)
    b_hhn = np.ascontiguousarray(b_hh[2 * H:].reshape(MN, 128).T)
    b_ihn = np.ascontiguousarray(b_ih[2 * H:].reshape(MN, 128).T)
    b_i2h_t = np.ascontiguousarray(b_i2h.reshape(KH, 128).T)
    b_out_bc = np.ascontiguousarray(np.broadcast_to(b_out, (128, V)))
    iota = np.ascontiguousarray(
        np.arange(V, dtype=np.float32).reshape(KV, 128).T)
    ones = np.ones((1, 128), dtype=np.float32)
    ident = np.eye(128, dtype=np.float32)

    M32 = np.ascontiguousarray((emb @ W_ih.T).astype(np.float32))
    g0 = ((sos[None, :] @ W_ih.T)[0] + b_ih).astype(np.float32)
    b_rz0 = np.ascontiguousarray((g0[:2 * H] + b_hh[:2 * H]).reshape(MRZ, 128).T)
    b_n0 = np.ascontiguousarray(g0[2 * H:].reshape(MN, 128).T)
    common = {
        "wi2hT": np.ascontiguousarray(W_i2h.T),
        "M": M32,
        "whhT": np.ascontiguousarray(W_hh.T),
        "woutT": np.ascontiguousarray(W_out.T),
        "b_rz": b_rz, "b_rz0": b_rz0, "b_n0": b_n0,
        "b_hhn": b_hhn, "b_ihn": b_ihn, "b_i2h": b_i2h_t,
        "b_out_bc": b_out_bc, "iota": iota,
        "ones": ones, "ident": ident,
    }
    in_maps = []
    for c in range(NCORES):
        m = dict(common)
        m["xT"] = np.ascontiguousarray(x[c * BL:(c + 1) * BL].T)
        in_maps.append(m)
    return in_maps


_cached = {}


def kernel(**inputs):
    if "prog" not in _cached:
        _cached["prog"] = build_program()
    nc = _cached["prog"]
    in_maps = make_in_maps(inputs)
    res = run_bass_kernel_spmd(nc, in_maps, core_ids=list(range(NCORES)))
    toks = np.concatenate([r["toks"] for r in res.results], axis=0)
    logits = np.concatenate([r["logits"] for r in res.results], axis=0)
    return toks.astype(np.int32), logits.astype(np.float32)


if __name__ == "__main__":
    t0 = time.time()
    build_program()
    print(f"build+compile: {time.time()-t0:.1f}s")
